# revision 61
# baseline (speedup 1.0000x reference)
"""Trainium2 Bass kernel for nn_BilinearFeedForward — n-split, 2-CC schedule.

Sharding: 8 cores = (batch b) x (n-half h).  Core 2b+h handles rows
n in [h*1024,(h+1)*1024) of batch b — the FLOP-minimal split
(12.9 GFLOP/core): K,V,Qr,Qi projections for its rows (bf16), partial
kv = K_h^T V_h + partial norm sums, pairwise AllReduces, then
out = q @ (diag(1/sk) kv diag(1/sv)) + bias.

Trace-driven schedule.  Measured on HW: each collective costs ~5-19us
pre-delay + ~25-38us for 1MB, all CCs serialize on the cc cores and
the one-time cc-core init varies 18-108us with cross-core launch
skew; every small DMA pays ~2-4us latency with ~2 in flight per
queue; only sync/scalar (HWDGE) and gpsimd (SWDGE) can issue DMAs;
the Tile scheduler hoists dependency-free DMAs to t=0 and interleaves
CC-dependent elementwise ops into engine queues where their waits
head-of-line-block everything behind them.  Hence:
  - only TWO collectives: the K-norm and per-half V-norm partial sums
    ride the two kv AllReduces as extra bf16 tail columns.
  - kv is interleaved with the V projection halves (V-eh0, kv-eh0 ->
    CC1, V-eh1, kv-eh1 -> CC2) so CC1 fires early and CC2 pipelines
    right behind it on the cc cores.
  - ALL post-CC math runs on scalar (+tiny gpsimd clamps), touched by
    no Q-phase-critical queue: skinv/svinv via Abs_reciprocal_sqrt
    activations, then 1/sk applied as 16 scalar Copy-activation row
    scales of the reduced kv, consumed only by the out phase (~40us
    of slack against CC/launch-skew variance); 1/sv + bias fold into
    the out scale on vector.
  - each CC half is staged into ONE contiguous SBUF tile; bounce
    writes and readbacks are split in half across two queues
    (per-queue DMA caps at ~80GB/s) to shorten the CC chain.
  - inputs are host-permuted so x halves and weight e-halves have
    8KB contiguous per-partition lines; the first window spreads the
    first K group's 2MB (xrt-h0 + wk-e0) across all three queues with
    xrt-h1 on the HWDGE queue tails; every other input DMA is
    data-dependency gated (1-elem tensor_copy into the DMA target) on
    K-phase progress; ~30 warmup matmuls on a memset tile ramp the PE
    p-state while the first DMAs land.

Engine streams (in-order each):
  tensor: warmup -> K -> V0 -> kv0 -> V1 -> kv1 -> Qr/Qi -> out
          (+ tiny fp32 partition-reduce matmuls for the norms)
  vector: psum copies + sq accumulate, kv staging copies, DMA gate
          copies, qr copies, q=qr*qi, out scale
  scalar: xrt-h0b/wk-e0b/bias/xit/wqi DMAs, squares, post-CC norm
          activations + kv row-scales, out DMAs
  sync:   xrt-h0a/wk-e0a DMAs, kv1 bounce, both readbacks, out DMAs
  gpsimd: xrt-h1/wk-e1/wv/wqr DMAs, kv0 bounce, both CCs, norm
          clamps, out DMAs
"""

import os
import sys
import numpy as np

for _p in ("/opt/trn_rl_repo", "/root/.axon_site/_ro/trn_rl_repo"):
    if _p not in sys.path and os.path.isdir(_p):
        sys.path.append(_p)

# Some images lack antenv.axon_hooks; bass_utils imports it unconditionally
# when BASS_TRACE is set.  Provide a degrade-to-no-trace shim if missing.
try:
    import antenv.axon_hooks  # noqa: F401
except Exception:
    import types

    try:
        import antenv

        _hooks = types.ModuleType("antenv.axon_hooks")
        _hooks._hook = None
        _hooks.get_axon_ntff_profile_hook = lambda: _hooks._hook

        def _set_hook(h):
            _hooks._hook = h

        _hooks.set_axon_ntff_profile_hook = _set_hook
        sys.modules["antenv.axon_hooks"] = _hooks
        antenv.axon_hooks = _hooks
    except Exception:
        pass

B, N, D = 4, 2048, 1024
N2 = N // 2       # rows per core
P = 128
DT = D // P       # 8 feature tiles
NT = N2 // P      # 8 n-tiles per core
EPS = 1e-5
HKV = DT * 512    # flat elems of one kv e-half (4096)

_CACHE = {}
LAST_EXEC_NS = None


def _build_bass():
    import concourse.bacc as bacc
    import concourse.tile as tile
    import concourse.mybir as mybir

    f32 = mybir.dt.float32
    bf16 = mybir.dt.bfloat16
    Act = mybir.ActivationFunctionType
    Alu = mybir.AluOpType

    RG = [[0, 1], [2, 3], [4, 5], [6, 7]]
    # CC payload per half: kv half (+ ssk and ssv-lo on CC1, ssv-hi on
    # CC2) — no separate norm collective
    W0 = HKV + DT + 4
    W1 = HKV + 4

    nc = bacc.Bacc()

    # x host-permuted to [p][nh][t][512], weights to [p][eh][t][512]:
    # every half is ONE 1MB DMA with 8KB contiguous per-partition lines.
    xrt_d = nc.dram_tensor("xrt", [P, 2 * DT * 512], bf16, kind="ExternalInput")
    xit_d = nc.dram_tensor("xit", [P, 2 * DT * 512], bf16, kind="ExternalInput")
    wk_d = nc.dram_tensor("wk", [P, 2 * DT * 512], bf16, kind="ExternalInput")
    wv_d = nc.dram_tensor("wv", [P, 2 * DT * 512], bf16, kind="ExternalInput")
    wqr_d = nc.dram_tensor("wqr", [P, 2 * DT * 512], bf16, kind="ExternalInput")
    wqi_d = nc.dram_tensor("wqi", [P, 2 * DT * 512], bf16, kind="ExternalInput")
    bias_d = nc.dram_tensor("bias", [D], f32, kind="ExternalInput")
    out_d = nc.dram_tensor("out_t", [D, N2], bf16, kind="ExternalOutput")

    xrt_r = xrt_d.rearrange("p (h t f) -> p h t f", h=2, t=DT)
    xit_r = xit_d.rearrange("p (h t f) -> p h t f", h=2, t=DT)
    wk_r = wk_d.rearrange("p (h t f) -> p h t f", h=2, t=DT)
    wv_r = wv_d.rearrange("p (h t f) -> p h t f", h=2, t=DT)
    wqr_r = wqr_d.rearrange("p (h t f) -> p h t f", h=2, t=DT)
    wqi_r = wqi_d.rearrange("p (h t f) -> p h t f", h=2, t=DT)
    bias_r = bias_d.rearrange("(t p) -> p t", p=P)
    out_r = out_d.rearrange("(t p) n -> p t n", p=P)

    with tile.TileContext(nc) as tc:
        with (
            tc.tile_pool(name="outer", bufs=1) as outer,
            tc.tile_pool(name="dram", bufs=1, space="DRAM") as dram,
        ):
            xrt_sb = outer.tile([P, 2, DT, 512], bf16, tag="xrt_sb")
            xit_sb = outer.tile([P, 2, DT, 512], bf16, tag="xit_sb")
            k_c0 = outer.tile([P, NT, 512], bf16, tag="k_c0")
            k_c1 = outer.tile([P, NT, 512], bf16, tag="k_c1")
            v_c0 = outer.tile([P, NT, 512], bf16, tag="v_c0")
            v_c1 = outer.tile([P, NT, 512], bf16, tag="v_c1")
            k_c = [k_c0, k_c1]
            v_c = [v_c0, v_c1]
            warm = outer.tile([P, 640], bf16, tag="warm")
            warm_sink = outer.tile([P, 1], f32, tag="warm_sink")
            # reduced kv halves (+norm tails) land here post-CC
            a_fl0 = outer.tile([P, W0], bf16, tag="a_fl0")
            a_fl1 = outer.tile([P, W1], bf16, tag="a_fl1")
            sqk = outer.tile([P, D], f32, tag="sqk")
            sqv = outer.tile([P, D], f32, tag="sqv")
            skinv = outer.tile([P, DT], f32, tag="skinv")
            svinv = outer.tile([P, DT], f32, tag="svinv")
            bias_pp = outer.tile([P, DT], f32, tag="bias_pp")
            zero32 = outer.tile([P, 1], f32, tag="zero32")
            ones32 = outer.tile([P, 1], f32, tag="ones32")

            nc.vector.memset(warm[:], 0.0)
            nc.vector.memset(zero32[:], 0.0)
            nc.vector.memset(ones32[:], 1.0)
            nc.vector.memset(sqk[:], 0.0)
            nc.vector.memset(sqv[:], 0.0)

            bb_i0 = dram.tile([P, W0], bf16, tag="bb_i0")
            bb_o0 = dram.tile([P, W0], bf16, tag="bb_o0")
            bb_i1 = dram.tile([P, W1], bf16, tag="bb_i1")
            bb_o1 = dram.tile([P, W1], bf16, tag="bb_o1")

            with tc.tile_pool(name="wq", bufs=1) as wq:
                wqr_sb = wq.tile([P, 2, DT, 512], bf16, tag="wqr_sb")
                wqi_sb = wq.tile([P, 2, DT, 512], bf16, tag="wqi_sb")

                with (
                    tc.tile_pool(name="wkv", bufs=1) as wkv,
                    tc.tile_pool(name="kvst", bufs=1) as kvst,
                    tc.tile_pool(name="sqt", bufs=4) as sqt,
                    tc.tile_pool(name="ps_kv", bufs=5, space="PSUM") as ps_kv,
                    tc.tile_pool(name="ps_n", bufs=1, space="PSUM") as ps_n,
                ):
                    wk_sb = wkv.tile([P, 2, DT, 512], bf16, tag="wk_sb")
                    wv_sb = wkv.tile([P, 2, DT, 512], bf16, tag="wv_sb")
                    # contiguous CC staging: kv half + norm tail
                    kv_st0 = kvst.tile([P, W0], bf16, tag="kv_st0")
                    kv_st1 = kvst.tile([P, W1], bf16, tag="kv_st1")

                    # PE p-state warmup: dummy matmuls on the memset tile
                    # fill the DMA lead-in window so the real stream
                    # starts at full clock with no ramp.
                    wps = ps_n.tile([P, 512], f32, tag="wps")
                    for _ in range(36):
                        nc.tensor.matmul(wps[:], warm[:, 0:128],
                                         warm[:, 128:640],
                                         start=True, stop=True)
                    nc.vector.tensor_copy(out=warm_sink[:], in_=wps[:, 0:1])

                    # Critical first window: per-queue DMA is capped at
                    # ~80GB/s no matter how many queues run, so the first
                    # group's 2MB (xrt-h0 + wk-e0) is split ~evenly across
                    # ALL THREE queues (first matmul ~18us instead of
                    # ~21); xrt-h1 rides the tails of the two HWDGE
                    # queues, landing just before group nt4 needs it.
                    # All other input DMAs are data-dependency gated on
                    # K-phase progress (the scheduler hoists anything
                    # without a dependency to t=0, recreating contention).
                    nc.sync.dma_start(out=xrt_sb[:, 0, 0:5], in_=xrt_r[:, 0, 0:5])
                    nc.scalar.dma_start(out=xrt_sb[:, 0, 5:8], in_=xrt_r[:, 0, 5:8])
                    nc.gpsimd.dma_start(out=wk_sb[:, 0, 2:8], in_=wk_r[:, 0, 2:8])
                    nc.scalar.dma_start(out=wk_sb[:, 0, 0:2], in_=wk_r[:, 0, 0:2])
                    nc.sync.dma_start(out=xrt_sb[:, 1, 0:4], in_=xrt_r[:, 1, 0:4])
                    nc.scalar.dma_start(out=bias_pp[:], in_=bias_r)
                    nc.scalar.dma_start(out=xrt_sb[:, 1, 4:8], in_=xrt_r[:, 1, 4:8])

                    def _gated(sb4d, h, dma_fn):
                        # 1-elem copy from the just-finished proj group's
                        # output into the DMA target creates a WAW dep that
                        # delays the DMA until that group completes.  The
                        # gate elem lives inside the half the DMA fills.
                        def g(dep_tile, nt):
                            nc.vector.tensor_copy(out=sb4d[:, h, 0, 0:1],
                                                  in_=dep_tile[:, nt, 0:1])
                            dma_fn()
                        return g

                    def src_ap(src_sb, nt, t):
                        j = (nt % 4) * P
                        return src_sb[:, nt // 4, t, j:j + P]

                    def proj_half(src_sb, w_sb, eh, dst, sq_acc, extras=None):
                        extras = list(extras or [])
                        esl = slice(eh * 512, (eh + 1) * 512)
                        for nt in range(NT):
                            kps = ps_kv.tile([P, 512], f32, tag="kps")
                            for t in range(DT):
                                nc.tensor.matmul(
                                    kps[:], src_ap(src_sb, nt, t),
                                    w_sb[:, eh, t, :],
                                    start=(t == 0), stop=(t == DT - 1))
                            nc.vector.tensor_copy(out=dst[:, nt, :], in_=kps[:])
                            sq = sqt.tile([P, 512], bf16, tag="sq")
                            nc.scalar.activation(out=sq[:], in_=kps[:],
                                                 func=Act.Square,
                                                 bias=zero32[:], scale=1.0)
                            nc.vector.tensor_add(out=sq_acc[:, esl],
                                                 in0=sq_acc[:, esl], in1=sq[:])
                            if extras:
                                extras.pop(0)(dst, nt)

                    def part_reduce(nps_t, sq_acc, blks, st_t, col):
                        # cross-partition reduce of sq_acc 128-blocks via
                        # matmul-with-ones into psum cols, then bf16 copy
                        # into the CC staging tail
                        for i, blk in enumerate(blks):
                            nc.tensor.matmul(nps_t[:, blk:blk + 1],
                                             sq_acc[:, blk * P:(blk + 1) * P],
                                             ones32[:], start=True, stop=True)
                        nc.vector.tensor_copy(
                            out=st_t[:, col:col + len(blks)],
                            in_=nps_t[:, blks[0]:blks[0] + len(blks)])

                    def kv_half(eh):
                        for dt in range(DT):
                            kc_h = k_c[dt // 4]
                            ksl = slice((dt % 4) * P, (dt % 4 + 1) * P)
                            aps = ps_kv.tile([P, 512], f32, tag="kps")
                            for nt2 in range(NT):
                                nc.tensor.matmul(
                                    aps[:], kc_h[:, nt2, ksl],
                                    v_c[eh][:, nt2, :],
                                    start=(nt2 == 0), stop=(nt2 == NT - 1))
                            st = kv_st0 if eh == 0 else kv_st1
                            nc.vector.tensor_copy(
                                out=st[:, dt * 512:(dt + 1) * 512], in_=aps[:])

                    # ---- K (both halves), norm partials into kv_st0 ----
                    keh0_extras = [
                        _gated(wk_sb, 1, lambda: nc.gpsimd.dma_start(
                            out=wk_sb[:, 1], in_=wk_r[:, 1])),
                        _gated(wv_sb, 0, lambda: nc.gpsimd.dma_start(
                            out=wv_sb[:, 0], in_=wv_r[:, 0])),
                        _gated(xit_sb, 0, lambda: nc.scalar.dma_start(
                            out=xit_sb[:, 0], in_=xit_r[:, 0])),
                        _gated(wv_sb, 1, lambda: nc.gpsimd.dma_start(
                            out=wv_sb[:, 1], in_=wv_r[:, 1])),
                        _gated(xit_sb, 1, lambda: nc.scalar.dma_start(
                            out=xit_sb[:, 1], in_=xit_r[:, 1])),
                        _gated(wqr_sb, 0, lambda: nc.gpsimd.dma_start(
                            out=wqr_sb[:, 0], in_=wqr_r[:, 0])),
                        _gated(wqr_sb, 1, lambda: nc.gpsimd.dma_start(
                            out=wqr_sb[:, 1], in_=wqr_r[:, 1])),
                    ]
                    keh1_extras = [
                        _gated(wqi_sb, 0, lambda: nc.scalar.dma_start(
                            out=wqi_sb[:, 0], in_=wqi_r[:, 0])),
                        _gated(wqi_sb, 1, lambda: nc.scalar.dma_start(
                            out=wqi_sb[:, 1], in_=wqi_r[:, 1])),
                    ]
                    nps = ps_n.tile([P, DT], f32, tag="nps")
                    npsv = ps_n.tile([P, DT], f32, tag="npsv")
                    proj_half(xrt_sb, wk_sb, 0, k_c0, sqk, extras=keh0_extras)
                    part_reduce(nps, sqk, [0, 1, 2, 3], kv_st0, HKV)
                    proj_half(xrt_sb, wk_sb, 1, k_c1, sqk, extras=keh1_extras)
                    part_reduce(nps, sqk, [4, 5, 6, 7], kv_st0, HKV + 4)

                    # ---- V-eh0, its norm partial, kv-eh0 -> CC1 ----
                    proj_half(xit_sb, wv_sb, 0, v_c0, sqv)
                    part_reduce(npsv, sqv, [0, 1, 2, 3], kv_st0, HKV + DT)
                    kv_half(0)
                    # split bounce writes/readbacks across two queues:
                    # per-queue DMA is ~80GB/s, so halving each 1MB
                    # transfer pulls the CC chain ~6us earlier per hop
                    nc.gpsimd.dma_start(out=bb_i0[:, :2048], in_=kv_st0[:, :2048])
                    nc.sync.dma_start(out=bb_i0[:, 2048:], in_=kv_st0[:, 2048:])
                    nc.gpsimd.collective_compute(
                        "AllReduce", Alu.add, replica_groups=RG,
                        ins=[bb_i0[:]], outs=[bb_o0[:]])

                    # ---- V-eh1, its norm partial, kv-eh1 -> CC2 ----
                    proj_half(xit_sb, wv_sb, 1, v_c1, sqv)
                    part_reduce(npsv, sqv, [4, 5, 6, 7], kv_st1, HKV)
                    kv_half(1)
                    nc.sync.dma_start(out=bb_i1[:, :2048], in_=kv_st1[:, :2048])
                    nc.scalar.dma_start(out=bb_i1[:, 2048:], in_=kv_st1[:, 2048:])
                    nc.gpsimd.collective_compute(
                        "AllReduce", Alu.add, replica_groups=RG,
                        ins=[bb_i1[:]], outs=[bb_o1[:]])
                    nc.sync.dma_start(out=a_fl0[:, :2048], in_=bb_o0[:, :2048])
                    nc.scalar.dma_start(out=a_fl0[:, 2048:], in_=bb_o0[:, 2048:])
                    nc.sync.dma_start(out=a_fl1[:, :2048], in_=bb_o1[:, :2048])
                    nc.scalar.dma_start(out=a_fl1[:, 2048:], in_=bb_o1[:, 2048:])

                # ------------- Phase C: Q^T then out^T = A^T Q^T -------------
                with (
                    tc.tile_pool(name="qtp", bufs=1) as qtp,
                    tc.tile_pool(name="qrp", bufs=3) as qrp,
                    tc.tile_pool(name="outp", bufs=4) as outp,
                    tc.tile_pool(name="ps_q", bufs=4, space="PSUM") as ps_q,
                    tc.tile_pool(name="ps_o", bufs=4, space="PSUM") as ps_o,
                ):
                    qt0 = qtp.tile([P, DT, 512], bf16, tag="qt0")
                    qt1 = qtp.tile([P, DT, 512], bf16, tag="qt1")
                    qt = [qt0, qt1]
                    for ch in range(2):
                        for dqt in range(DT):
                            qh = dqt // 4
                            qsl = slice((dqt % 4) * P, (dqt % 4 + 1) * P)
                            qrps = ps_q.tile([P, 512], f32, tag="qps")
                            for t in range(DT):
                                nc.tensor.matmul(qrps[:],
                                                 wqr_sb[:, qh, t, qsl],
                                                 xrt_sb[:, ch, t, :],
                                                 start=(t == 0), stop=(t == DT - 1))
                            qr_sb = qrp.tile([P, 512], f32, tag="qr_sb")
                            nc.vector.tensor_copy(out=qr_sb[:], in_=qrps[:])
                            qips = ps_q.tile([P, 512], f32, tag="qps")
                            for t in range(DT):
                                nc.tensor.matmul(qips[:],
                                                 wqi_sb[:, qh, t, qsl],
                                                 xit_sb[:, ch, t, :],
                                                 start=(t == 0), stop=(t == DT - 1))
                            nc.vector.tensor_tensor(
                                out=qt[ch][:, dqt, :], in0=qips[:],
                                in1=qr_sb[:], op=Alu.mult)

                    # post-CC norm processing, entirely on scalar+gpsimd
                    # (idle mid-kernel): any op waiting on a CC readback
                    # head-of-line-blocks its engine queue, and vector
                    # carries the q staging.  1/sk is applied as row
                    # scales on the reduced kv (consumed only by the out
                    # phase, maximizing slack against cross-core launch
                    # skew in the collectives), 1/sv via the out scale.
                    nc.scalar.activation(out=skinv[:],
                                         in_=a_fl0[:, HKV:HKV + DT],
                                         func=Act.Abs_reciprocal_sqrt,
                                         bias=zero32[:], scale=1.0)
                    nc.gpsimd.tensor_scalar_min(skinv[:], skinv[:], 1.0 / EPS)
                    nc.scalar.activation(out=svinv[:, 0:4],
                                         in_=a_fl0[:, HKV + DT:],
                                         func=Act.Abs_reciprocal_sqrt,
                                         bias=zero32[:], scale=1.0)
                    nc.scalar.activation(out=svinv[:, 4:8],
                                         in_=a_fl1[:, HKV:],
                                         func=Act.Abs_reciprocal_sqrt,
                                         bias=zero32[:], scale=1.0)
                    nc.gpsimd.tensor_scalar_min(svinv[:], svinv[:], 1.0 / EPS)
                    for a_fl in (a_fl0, a_fl1):
                        for dt in range(DT):
                            dsl = slice(dt * 512, (dt + 1) * 512)
                            nc.scalar.activation(
                                out=a_fl[:, dsl], in_=a_fl[:, dsl],
                                func=Act.Copy, bias=0.0,
                                scale=skinv[:, dt:dt + 1])

                    for et in range(DT):
                        a_fl = a_fl0 if et < 4 else a_fl1
                        ecol = (et % 4) * P
                        o_sb = outp.tile([P, N2], bf16, tag="o_sb")
                        for ch in range(2):
                            nsl = slice(ch * 512, (ch + 1) * 512)
                            ops = ps_o.tile([P, 512], f32, tag="ops")
                            for dt in range(DT):
                                nc.tensor.matmul(
                                    ops[:],
                                    a_fl[:, dt * 512 + ecol:dt * 512 + ecol + P],
                                    qt[ch][:, dt, :],
                                    start=(dt == 0), stop=(dt == DT - 1))
                            if et < DT - 1:
                                nc.vector.tensor_scalar(
                                    out=o_sb[:, nsl], in0=ops[:],
                                    scalar1=svinv[:, et:et + 1],
                                    scalar2=bias_pp[:, et:et + 1],
                                    op0=Alu.mult, op1=Alu.add)
                            else:
                                for qh in range(2):
                                    q2 = slice(ch * 512 + qh * 256,
                                               ch * 512 + (qh + 1) * 256)
                                    nc.vector.tensor_scalar(
                                        out=o_sb[:, q2],
                                        in0=ops[:, qh * 256:(qh + 1) * 256],
                                        scalar1=svinv[:, et:et + 1],
                                        scalar2=bias_pp[:, et:et + 1],
                                        op0=Alu.mult, op1=Alu.add)
                            if et == DT - 1:
                                # quarter the final tile's stores so the
                                # post-stream tail transfer is 2x64KB on
                                # the two HWDGE queues in parallel
                                for qh in range(2):
                                    q2 = slice(ch * 512 + qh * 256,
                                               ch * 512 + (qh + 1) * 256)
                                    eng = (nc.sync, nc.scalar)[qh]
                                    eng.dma_start(out=out_r[:, et, q2],
                                                  in_=o_sb[:, q2])
                        if et < DT - 1:
                            eng = (nc.scalar, nc.sync, nc.gpsimd)[et % 3]
                            eng.dma_start(out=out_r[:, et, :], in_=o_sb[:])

    nc.finalize()
    return nc


def kernel(x_real, x_imag, w_query_real, w_query_imag, w_key, w_value, bias):
    global LAST_EXEC_NS
    from concourse.bass_utils import run_bass_kernel_spmd
    import ml_dtypes

    bfdt = ml_dtypes.bfloat16

    x_real = np.asarray(x_real, dtype=np.float32)
    x_imag = np.asarray(x_imag, dtype=np.float32)

    def _wperm(w):
        # [D, D] (d, e) -> [p][eh][t][f] (d = t*128+p, e = eh*512+f):
        # each e-half is one contiguous-line 1MB DMA
        w = np.asarray(w, dtype=np.float32).astype(bfdt)
        return np.ascontiguousarray(
            w.reshape(DT, P, 2, 512).transpose(1, 2, 0, 3).reshape(P, -1))

    def _xperm(x):
        # [N2, D] (n, d) -> [p][nh][t][j] (d = t*128+p, n = nh*512+j)
        return np.ascontiguousarray(
            x.astype(bfdt).reshape(2, 512, DT, P)
            .transpose(3, 0, 2, 1).reshape(P, -1))

    wqr = _wperm(w_query_real)
    wqi = _wperm(w_query_imag)
    wk = _wperm(w_key)
    wv = _wperm(w_value)
    bias = np.ascontiguousarray(np.asarray(bias, dtype=np.float32))

    nc = _CACHE.get("nc")
    if nc is None:
        nc = _build_bass()
        _CACHE["nc"] = nc

    in_maps = []
    for c in range(8):
        b, h = c // 2, c % 2
        nsl = slice(h * N2, (h + 1) * N2)
        in_maps.append({
            "xrt": _xperm(x_real[b, nsl]),
            "xit": _xperm(x_imag[b, nsl]),
            "wk": wk, "wv": wv, "wqr": wqr, "wqi": wqi,
            "bias": bias,
        })

    res = run_bass_kernel_spmd(nc, in_maps, list(range(8)))
    LAST_EXEC_NS = res.exec_time_ns

    out = np.empty((B, N, D), dtype=np.float32)
    for c in range(8):
        b, h = c // 2, c % 2
        out[b, h * N2:(h + 1) * N2, :] = \
            np.asarray(res.results[c]["out_t"]).astype(np.float32).T
    return out


# revision 62
# speedup vs baseline: 1.0031x; 1.0031x over previous
"""Trainium2 Bass kernel for nn_BilinearFeedForward — n-split, 2-CC schedule.

Sharding: 8 cores = (batch b) x (n-half h).  Core 2b+h handles rows
n in [h*1024,(h+1)*1024) of batch b — the FLOP-minimal split
(12.9 GFLOP/core): K,V,Qr,Qi projections for its rows (bf16), partial
kv = K_h^T V_h + partial norm sums, pairwise AllReduces, then
out = q @ (diag(1/sk) kv diag(1/sv)) + bias.

Trace-driven schedule.  Measured on HW: each collective costs ~5-19us
pre-delay + ~25-38us for 1MB, all CCs serialize on the cc cores and
the one-time cc-core init varies 18-108us with cross-core launch
skew; every small DMA pays ~2-4us latency with ~2 in flight per
queue; only sync/scalar (HWDGE) and gpsimd (SWDGE) can issue DMAs;
the Tile scheduler hoists dependency-free DMAs to t=0 and interleaves
CC-dependent elementwise ops into engine queues where their waits
head-of-line-block everything behind them.  Hence:
  - only TWO collectives: the K-norm and per-half V-norm partial sums
    ride the two kv AllReduces as extra bf16 tail columns.
  - kv is interleaved with the V projection halves (V-eh0, kv-eh0 ->
    CC1, V-eh1, kv-eh1 -> CC2) so CC1 fires early and CC2 pipelines
    right behind it on the cc cores.
  - ALL post-CC math runs on scalar (+tiny gpsimd clamps), touched by
    no Q-phase-critical queue: skinv/svinv via Abs_reciprocal_sqrt
    activations, then 1/sk applied as 16 scalar Copy-activation row
    scales of the reduced kv, consumed only by the out phase (~40us
    of slack against CC/launch-skew variance); 1/sv + bias fold into
    the out scale on vector.
  - each CC half is staged into ONE contiguous SBUF tile; bounce
    writes and readbacks are split in half across two queues
    (per-queue DMA caps at ~80GB/s) to shorten the CC chain.
  - inputs are host-permuted so x halves and weight e-halves have
    8KB contiguous per-partition lines; the first window spreads the
    first K group's 2MB (xrt-h0 + wk-e0) across all three queues with
    xrt-h1 on the HWDGE queue tails; every other input DMA is
    data-dependency gated (1-elem tensor_copy into the DMA target) on
    K-phase progress; ~30 warmup matmuls on a memset tile ramp the PE
    p-state while the first DMAs land.

Engine streams (in-order each):
  tensor: warmup -> K -> V0 -> kv0 -> V1 -> kv1 -> Qr/Qi -> out
          (+ tiny fp32 partition-reduce matmuls for the norms)
  vector: psum copies + sq accumulate, kv staging copies, DMA gate
          copies, qr copies, q=qr*qi, out scale
  scalar: xrt-h0b/wk-e0b/bias/xit/wqi DMAs, squares, post-CC norm
          activations + kv row-scales, out DMAs
  sync:   xrt-h0a/wk-e0a DMAs, kv1 bounce, both readbacks, out DMAs
  gpsimd: xrt-h1/wk-e1/wv/wqr DMAs, kv0 bounce, both CCs, norm
          clamps, out DMAs
"""

import os
import sys
import numpy as np

for _p in ("/opt/trn_rl_repo", "/root/.axon_site/_ro/trn_rl_repo"):
    if _p not in sys.path and os.path.isdir(_p):
        sys.path.append(_p)

# Some images lack antenv.axon_hooks; bass_utils imports it unconditionally
# when BASS_TRACE is set.  Provide a degrade-to-no-trace shim if missing.
try:
    import antenv.axon_hooks  # noqa: F401
except Exception:
    import types

    try:
        import antenv

        _hooks = types.ModuleType("antenv.axon_hooks")
        _hooks._hook = None
        _hooks.get_axon_ntff_profile_hook = lambda: _hooks._hook

        def _set_hook(h):
            _hooks._hook = h

        _hooks.set_axon_ntff_profile_hook = _set_hook
        sys.modules["antenv.axon_hooks"] = _hooks
        antenv.axon_hooks = _hooks
    except Exception:
        pass

B, N, D = 4, 2048, 1024
N2 = N // 2       # rows per core
P = 128
DT = D // P       # 8 feature tiles
NT = N2 // P      # 8 n-tiles per core
EPS = 1e-5
HKV = DT * 512    # flat elems of one kv e-half (4096)

_CACHE = {}
LAST_EXEC_NS = None


def _build_bass():
    import concourse.bacc as bacc
    import concourse.tile as tile
    import concourse.mybir as mybir

    f32 = mybir.dt.float32
    bf16 = mybir.dt.bfloat16
    Act = mybir.ActivationFunctionType
    Alu = mybir.AluOpType

    RG = [[0, 1], [2, 3], [4, 5], [6, 7]]
    # CC payload per half: kv half (+ ssk and ssv-lo on CC1, ssv-hi on
    # CC2) — no separate norm collective
    W0 = HKV + DT + 4
    W1 = HKV + 4

    nc = bacc.Bacc()

    # x host-permuted to [p][nh][t][512], weights to [p][eh][t][512]:
    # every half is ONE 1MB DMA with 8KB contiguous per-partition lines.
    xrt_d = nc.dram_tensor("xrt", [P, 2 * DT * 512], bf16, kind="ExternalInput")
    xit_d = nc.dram_tensor("xit", [P, 2 * DT * 512], bf16, kind="ExternalInput")
    wk_d = nc.dram_tensor("wk", [P, 2 * DT * 512], bf16, kind="ExternalInput")
    wv_d = nc.dram_tensor("wv", [P, 2 * DT * 512], bf16, kind="ExternalInput")
    wqr_d = nc.dram_tensor("wqr", [P, 2 * DT * 512], bf16, kind="ExternalInput")
    wqi_d = nc.dram_tensor("wqi", [P, 2 * DT * 512], bf16, kind="ExternalInput")
    bias_d = nc.dram_tensor("bias", [D], f32, kind="ExternalInput")
    out_d = nc.dram_tensor("out_t", [D, N2], bf16, kind="ExternalOutput")

    xrt_r = xrt_d.rearrange("p (h t f) -> p h t f", h=2, t=DT)
    xit_r = xit_d.rearrange("p (h t f) -> p h t f", h=2, t=DT)
    wk_r = wk_d.rearrange("p (h t f) -> p h t f", h=2, t=DT)
    wv_r = wv_d.rearrange("p (h t f) -> p h t f", h=2, t=DT)
    wqr_r = wqr_d.rearrange("p (h t f) -> p h t f", h=2, t=DT)
    wqi_r = wqi_d.rearrange("p (h t f) -> p h t f", h=2, t=DT)
    bias_r = bias_d.rearrange("(t p) -> p t", p=P)
    out_r = out_d.rearrange("(t p) n -> p t n", p=P)

    with tile.TileContext(nc) as tc:
        with (
            tc.tile_pool(name="outer", bufs=1) as outer,
            tc.tile_pool(name="dram", bufs=1, space="DRAM") as dram,
        ):
            xrt_sb = outer.tile([P, 2, DT, 512], bf16, tag="xrt_sb")
            xit_sb = outer.tile([P, 2, DT, 512], bf16, tag="xit_sb")
            k_c0 = outer.tile([P, NT, 512], bf16, tag="k_c0")
            k_c1 = outer.tile([P, NT, 512], bf16, tag="k_c1")
            v_c0 = outer.tile([P, NT, 512], bf16, tag="v_c0")
            v_c1 = outer.tile([P, NT, 512], bf16, tag="v_c1")
            k_c = [k_c0, k_c1]
            v_c = [v_c0, v_c1]
            warm = outer.tile([P, 640], bf16, tag="warm")
            warm_sink = outer.tile([P, 1], f32, tag="warm_sink")
            # reduced kv halves (+norm tails) land here post-CC
            a_fl0 = outer.tile([P, W0], bf16, tag="a_fl0")
            a_fl1 = outer.tile([P, W1], bf16, tag="a_fl1")
            sqk = outer.tile([P, D], f32, tag="sqk")
            sqv = outer.tile([P, D], f32, tag="sqv")
            skinv = outer.tile([P, DT], f32, tag="skinv")
            svinv = outer.tile([P, DT], f32, tag="svinv")
            bias_pp = outer.tile([P, DT], f32, tag="bias_pp")
            zero32 = outer.tile([P, 1], f32, tag="zero32")
            ones32 = outer.tile([P, 1], f32, tag="ones32")

            nc.vector.memset(warm[:], 0.0)
            nc.vector.memset(zero32[:], 0.0)
            nc.vector.memset(ones32[:], 1.0)
            nc.vector.memset(sqk[:], 0.0)
            nc.vector.memset(sqv[:], 0.0)

            bb_i0 = dram.tile([P, W0], bf16, tag="bb_i0")
            bb_o0 = dram.tile([P, W0], bf16, tag="bb_o0")
            bb_i1 = dram.tile([P, W1], bf16, tag="bb_i1")
            bb_o1 = dram.tile([P, W1], bf16, tag="bb_o1")

            with tc.tile_pool(name="wq", bufs=1) as wq:
                wqr_sb = wq.tile([P, 2, DT, 512], bf16, tag="wqr_sb")
                wqi_sb = wq.tile([P, 2, DT, 512], bf16, tag="wqi_sb")

                with (
                    tc.tile_pool(name="wkv", bufs=1) as wkv,
                    tc.tile_pool(name="kvst", bufs=1) as kvst,
                    tc.tile_pool(name="sqt", bufs=4) as sqt,
                    tc.tile_pool(name="ps_kv", bufs=5, space="PSUM") as ps_kv,
                    tc.tile_pool(name="ps_n", bufs=1, space="PSUM") as ps_n,
                ):
                    wk_sb = wkv.tile([P, 2, DT, 512], bf16, tag="wk_sb")
                    wv_sb = wkv.tile([P, 2, DT, 512], bf16, tag="wv_sb")
                    # contiguous CC staging: kv half + norm tail
                    kv_st0 = kvst.tile([P, W0], bf16, tag="kv_st0")
                    kv_st1 = kvst.tile([P, W1], bf16, tag="kv_st1")

                    # PE p-state warmup: dummy matmuls on the memset tile
                    # fill the DMA lead-in window so the real stream
                    # starts at full clock with no ramp.
                    wps = ps_n.tile([P, 512], f32, tag="wps")
                    for _ in range(30):
                        nc.tensor.matmul(wps[:], warm[:, 0:128],
                                         warm[:, 128:640],
                                         start=True, stop=True)
                    nc.vector.tensor_copy(out=warm_sink[:], in_=wps[:, 0:1])

                    # Critical first window: per-queue DMA is capped at
                    # ~80GB/s no matter how many queues run, so the first
                    # group's 2MB (xrt-h0 + wk-e0) is split ~evenly across
                    # ALL THREE queues (first matmul ~18us instead of
                    # ~21); xrt-h1 rides the tails of the two HWDGE
                    # queues, landing just before group nt4 needs it.
                    # All other input DMAs are data-dependency gated on
                    # K-phase progress (the scheduler hoists anything
                    # without a dependency to t=0, recreating contention).
                    nc.sync.dma_start(out=xrt_sb[:, 0, 0:5], in_=xrt_r[:, 0, 0:5])
                    nc.scalar.dma_start(out=xrt_sb[:, 0, 5:8], in_=xrt_r[:, 0, 5:8])
                    nc.gpsimd.dma_start(out=wk_sb[:, 0, 2:8], in_=wk_r[:, 0, 2:8])
                    nc.scalar.dma_start(out=wk_sb[:, 0, 0:2], in_=wk_r[:, 0, 0:2])
                    nc.sync.dma_start(out=xrt_sb[:, 1, 0:4], in_=xrt_r[:, 1, 0:4])
                    nc.scalar.dma_start(out=bias_pp[:], in_=bias_r)
                    nc.scalar.dma_start(out=xrt_sb[:, 1, 4:8], in_=xrt_r[:, 1, 4:8])

                    def _gated(sb4d, h, dma_fn):
                        # 1-elem copy from the just-finished proj group's
                        # output into the DMA target creates a WAW dep that
                        # delays the DMA until that group completes.  The
                        # gate elem lives inside the half the DMA fills.
                        def g(dep_tile, nt):
                            nc.vector.tensor_copy(out=sb4d[:, h, 0, 0:1],
                                                  in_=dep_tile[:, nt, 0:1])
                            dma_fn()
                        return g

                    def src_ap(src_sb, nt, t):
                        j = (nt % 4) * P
                        return src_sb[:, nt // 4, t, j:j + P]

                    def proj_half(src_sb, w_sb, eh, dst, sq_acc, extras=None):
                        extras = list(extras or [])
                        esl = slice(eh * 512, (eh + 1) * 512)
                        for nt in range(NT):
                            kps = ps_kv.tile([P, 512], f32, tag="kps")
                            for t in range(DT):
                                nc.tensor.matmul(
                                    kps[:], src_ap(src_sb, nt, t),
                                    w_sb[:, eh, t, :],
                                    start=(t == 0), stop=(t == DT - 1))
                            nc.vector.tensor_copy(out=dst[:, nt, :], in_=kps[:])
                            sq = sqt.tile([P, 512], bf16, tag="sq")
                            nc.scalar.activation(out=sq[:], in_=kps[:],
                                                 func=Act.Square,
                                                 bias=zero32[:], scale=1.0)
                            nc.vector.tensor_add(out=sq_acc[:, esl],
                                                 in0=sq_acc[:, esl], in1=sq[:])
                            if extras:
                                extras.pop(0)(dst, nt)

                    def part_reduce(nps_t, sq_acc, blks, st_t, col):
                        # cross-partition reduce of sq_acc 128-blocks via
                        # matmul-with-ones into psum cols, then bf16 copy
                        # into the CC staging tail
                        for i, blk in enumerate(blks):
                            nc.tensor.matmul(nps_t[:, blk:blk + 1],
                                             sq_acc[:, blk * P:(blk + 1) * P],
                                             ones32[:], start=True, stop=True)
                        nc.vector.tensor_copy(
                            out=st_t[:, col:col + len(blks)],
                            in_=nps_t[:, blks[0]:blks[0] + len(blks)])

                    def kv_half(eh):
                        for dt in range(DT):
                            kc_h = k_c[dt // 4]
                            ksl = slice((dt % 4) * P, (dt % 4 + 1) * P)
                            aps = ps_kv.tile([P, 512], f32, tag="kps")
                            for nt2 in range(NT):
                                nc.tensor.matmul(
                                    aps[:], kc_h[:, nt2, ksl],
                                    v_c[eh][:, nt2, :],
                                    start=(nt2 == 0), stop=(nt2 == NT - 1))
                            st = kv_st0 if eh == 0 else kv_st1
                            nc.vector.tensor_copy(
                                out=st[:, dt * 512:(dt + 1) * 512], in_=aps[:])

                    # ---- K (both halves), norm partials into kv_st0 ----
                    keh0_extras = [
                        _gated(wk_sb, 1, lambda: nc.gpsimd.dma_start(
                            out=wk_sb[:, 1], in_=wk_r[:, 1])),
                        _gated(wv_sb, 0, lambda: nc.gpsimd.dma_start(
                            out=wv_sb[:, 0], in_=wv_r[:, 0])),
                        _gated(xit_sb, 0, lambda: nc.scalar.dma_start(
                            out=xit_sb[:, 0], in_=xit_r[:, 0])),
                        _gated(wv_sb, 1, lambda: nc.gpsimd.dma_start(
                            out=wv_sb[:, 1], in_=wv_r[:, 1])),
                        _gated(xit_sb, 1, lambda: nc.scalar.dma_start(
                            out=xit_sb[:, 1], in_=xit_r[:, 1])),
                        _gated(wqr_sb, 0, lambda: nc.gpsimd.dma_start(
                            out=wqr_sb[:, 0], in_=wqr_r[:, 0])),
                        _gated(wqr_sb, 1, lambda: nc.gpsimd.dma_start(
                            out=wqr_sb[:, 1], in_=wqr_r[:, 1])),
                    ]
                    keh1_extras = [
                        _gated(wqi_sb, 0, lambda: nc.scalar.dma_start(
                            out=wqi_sb[:, 0], in_=wqi_r[:, 0])),
                        _gated(wqi_sb, 1, lambda: nc.scalar.dma_start(
                            out=wqi_sb[:, 1], in_=wqi_r[:, 1])),
                    ]
                    nps = ps_n.tile([P, DT], f32, tag="nps")
                    npsv = ps_n.tile([P, DT], f32, tag="npsv")
                    proj_half(xrt_sb, wk_sb, 0, k_c0, sqk, extras=keh0_extras)
                    part_reduce(nps, sqk, [0, 1, 2, 3], kv_st0, HKV)
                    proj_half(xrt_sb, wk_sb, 1, k_c1, sqk, extras=keh1_extras)
                    part_reduce(nps, sqk, [4, 5, 6, 7], kv_st0, HKV + 4)

                    # ---- V-eh0, its norm partial, kv-eh0 -> CC1 ----
                    proj_half(xit_sb, wv_sb, 0, v_c0, sqv)
                    part_reduce(npsv, sqv, [0, 1, 2, 3], kv_st0, HKV + DT)
                    kv_half(0)
                    # split bounce writes/readbacks across two queues:
                    # per-queue DMA is ~80GB/s, so halving each 1MB
                    # transfer pulls the CC chain ~6us earlier per hop
                    nc.gpsimd.dma_start(out=bb_i0[:, :2048], in_=kv_st0[:, :2048])
                    nc.sync.dma_start(out=bb_i0[:, 2048:], in_=kv_st0[:, 2048:])
                    nc.gpsimd.collective_compute(
                        "AllReduce", Alu.add, replica_groups=RG,
                        ins=[bb_i0[:]], outs=[bb_o0[:]])

                    # ---- V-eh1, its norm partial, kv-eh1 -> CC2 ----
                    proj_half(xit_sb, wv_sb, 1, v_c1, sqv)
                    part_reduce(npsv, sqv, [4, 5, 6, 7], kv_st1, HKV)
                    kv_half(1)
                    nc.sync.dma_start(out=bb_i1[:, :2048], in_=kv_st1[:, :2048])
                    nc.scalar.dma_start(out=bb_i1[:, 2048:], in_=kv_st1[:, 2048:])
                    nc.gpsimd.collective_compute(
                        "AllReduce", Alu.add, replica_groups=RG,
                        ins=[bb_i1[:]], outs=[bb_o1[:]])
                    nc.sync.dma_start(out=a_fl0[:, :2048], in_=bb_o0[:, :2048])
                    nc.scalar.dma_start(out=a_fl0[:, 2048:], in_=bb_o0[:, 2048:])
                    nc.sync.dma_start(out=a_fl1[:, :2048], in_=bb_o1[:, :2048])
                    nc.scalar.dma_start(out=a_fl1[:, 2048:], in_=bb_o1[:, 2048:])

                # ------------- Phase C: Q^T then out^T = A^T Q^T -------------
                with (
                    tc.tile_pool(name="qtp", bufs=1) as qtp,
                    tc.tile_pool(name="qrp", bufs=3) as qrp,
                    tc.tile_pool(name="outp", bufs=4) as outp,
                    tc.tile_pool(name="ps_q", bufs=4, space="PSUM") as ps_q,
                    tc.tile_pool(name="ps_o", bufs=4, space="PSUM") as ps_o,
                ):
                    qt0 = qtp.tile([P, DT, 512], bf16, tag="qt0")
                    qt1 = qtp.tile([P, DT, 512], bf16, tag="qt1")
                    qt = [qt0, qt1]
                    for ch in range(2):
                        for dqt in range(DT):
                            qh = dqt // 4
                            qsl = slice((dqt % 4) * P, (dqt % 4 + 1) * P)
                            qrps = ps_q.tile([P, 512], f32, tag="qps")
                            for t in range(DT):
                                nc.tensor.matmul(qrps[:],
                                                 wqr_sb[:, qh, t, qsl],
                                                 xrt_sb[:, ch, t, :],
                                                 start=(t == 0), stop=(t == DT - 1))
                            qr_sb = qrp.tile([P, 512], f32, tag="qr_sb")
                            nc.vector.tensor_copy(out=qr_sb[:], in_=qrps[:])
                            qips = ps_q.tile([P, 512], f32, tag="qps")
                            for t in range(DT):
                                nc.tensor.matmul(qips[:],
                                                 wqi_sb[:, qh, t, qsl],
                                                 xit_sb[:, ch, t, :],
                                                 start=(t == 0), stop=(t == DT - 1))
                            nc.vector.tensor_tensor(
                                out=qt[ch][:, dqt, :], in0=qips[:],
                                in1=qr_sb[:], op=Alu.mult)

                    # post-CC norm processing, entirely on scalar+gpsimd
                    # (idle mid-kernel): any op waiting on a CC readback
                    # head-of-line-blocks its engine queue, and vector
                    # carries the q staging.  1/sk is applied as row
                    # scales on the reduced kv (consumed only by the out
                    # phase, maximizing slack against cross-core launch
                    # skew in the collectives), 1/sv via the out scale.
                    nc.scalar.activation(out=skinv[:],
                                         in_=a_fl0[:, HKV:HKV + DT],
                                         func=Act.Abs_reciprocal_sqrt,
                                         bias=zero32[:], scale=1.0)
                    nc.gpsimd.tensor_scalar_min(skinv[:], skinv[:], 1.0 / EPS)
                    nc.scalar.activation(out=svinv[:, 0:4],
                                         in_=a_fl0[:, HKV + DT:],
                                         func=Act.Abs_reciprocal_sqrt,
                                         bias=zero32[:], scale=1.0)
                    nc.scalar.activation(out=svinv[:, 4:8],
                                         in_=a_fl1[:, HKV:],
                                         func=Act.Abs_reciprocal_sqrt,
                                         bias=zero32[:], scale=1.0)
                    nc.gpsimd.tensor_scalar_min(svinv[:], svinv[:], 1.0 / EPS)
                    for a_fl in (a_fl0, a_fl1):
                        for dt in range(DT):
                            dsl = slice(dt * 512, (dt + 1) * 512)
                            nc.scalar.activation(
                                out=a_fl[:, dsl], in_=a_fl[:, dsl],
                                func=Act.Copy, bias=0.0,
                                scale=skinv[:, dt:dt + 1])

                    for et in range(DT):
                        a_fl = a_fl0 if et < 4 else a_fl1
                        ecol = (et % 4) * P
                        o_sb = outp.tile([P, N2], bf16, tag="o_sb")
                        for ch in range(2):
                            nsl = slice(ch * 512, (ch + 1) * 512)
                            ops = ps_o.tile([P, 512], f32, tag="ops")
                            for dt in range(DT):
                                nc.tensor.matmul(
                                    ops[:],
                                    a_fl[:, dt * 512 + ecol:dt * 512 + ecol + P],
                                    qt[ch][:, dt, :],
                                    start=(dt == 0), stop=(dt == DT - 1))
                            if et < DT - 1:
                                nc.vector.tensor_scalar(
                                    out=o_sb[:, nsl], in0=ops[:],
                                    scalar1=svinv[:, et:et + 1],
                                    scalar2=bias_pp[:, et:et + 1],
                                    op0=Alu.mult, op1=Alu.add)
                            else:
                                for qh in range(2):
                                    q2 = slice(ch * 512 + qh * 256,
                                               ch * 512 + (qh + 1) * 256)
                                    nc.vector.tensor_scalar(
                                        out=o_sb[:, q2],
                                        in0=ops[:, qh * 256:(qh + 1) * 256],
                                        scalar1=svinv[:, et:et + 1],
                                        scalar2=bias_pp[:, et:et + 1],
                                        op0=Alu.mult, op1=Alu.add)
                            if et == DT - 1:
                                # quarter the final tile's stores so the
                                # post-stream tail transfer is 2x64KB on
                                # the two HWDGE queues in parallel
                                for qh in range(2):
                                    q2 = slice(ch * 512 + qh * 256,
                                               ch * 512 + (qh + 1) * 256)
                                    eng = (nc.sync, nc.scalar)[qh]
                                    eng.dma_start(out=out_r[:, et, q2],
                                                  in_=o_sb[:, q2])
                        if et < DT - 1:
                            eng = (nc.scalar, nc.sync, nc.gpsimd)[et % 3]
                            eng.dma_start(out=out_r[:, et, :], in_=o_sb[:])

    nc.finalize()
    return nc


def kernel(x_real, x_imag, w_query_real, w_query_imag, w_key, w_value, bias):
    global LAST_EXEC_NS
    from concourse.bass_utils import run_bass_kernel_spmd
    import ml_dtypes

    bfdt = ml_dtypes.bfloat16

    x_real = np.asarray(x_real, dtype=np.float32)
    x_imag = np.asarray(x_imag, dtype=np.float32)

    def _wperm(w):
        # [D, D] (d, e) -> [p][eh][t][f] (d = t*128+p, e = eh*512+f):
        # each e-half is one contiguous-line 1MB DMA
        w = np.asarray(w, dtype=np.float32).astype(bfdt)
        return np.ascontiguousarray(
            w.reshape(DT, P, 2, 512).transpose(1, 2, 0, 3).reshape(P, -1))

    def _xperm(x):
        # [N2, D] (n, d) -> [p][nh][t][j] (d = t*128+p, n = nh*512+j)
        return np.ascontiguousarray(
            x.astype(bfdt).reshape(2, 512, DT, P)
            .transpose(3, 0, 2, 1).reshape(P, -1))

    wqr = _wperm(w_query_real)
    wqi = _wperm(w_query_imag)
    wk = _wperm(w_key)
    wv = _wperm(w_value)
    bias = np.ascontiguousarray(np.asarray(bias, dtype=np.float32))

    nc = _CACHE.get("nc")
    if nc is None:
        nc = _build_bass()
        _CACHE["nc"] = nc

    in_maps = []
    for c in range(8):
        b, h = c // 2, c % 2
        nsl = slice(h * N2, (h + 1) * N2)
        in_maps.append({
            "xrt": _xperm(x_real[b, nsl]),
            "xit": _xperm(x_imag[b, nsl]),
            "wk": wk, "wv": wv, "wqr": wqr, "wqi": wqi,
            "bias": bias,
        })

    res = run_bass_kernel_spmd(nc, in_maps, list(range(8)))
    LAST_EXEC_NS = res.exec_time_ns

    out = np.empty((B, N, D), dtype=np.float32)
    for c in range(8):
        b, h = c // 2, c % 2
        out[b, h * N2:(h + 1) * N2, :] = \
            np.asarray(res.results[c]["out_t"]).astype(np.float32).T
    return out
